# revision 25
# baseline (speedup 1.0000x reference)
"""Trainium2 8-core kernel for the GConvGRU-style GNN message-passing net.

Reference computation (N=100000 nodes, E=400000 edges, y = out[:50000]):
    deg  = indeg(dst) + 1;  dinv = rsqrt(deg)
    xs   = D^-1/2 (A + I) D^-1/2 x          # [N, 32] normalized aggregation
    cz   = xs @ Wz + bz ; ch = xs @ Wh + bh # (H == 0 for this problem)
    Z    = sigmoid(cz @ Lz_top + Lz_b); H~ = tanh(ch @ Lh_top + Lh_b)
    Hn   = (1 - Z) * H~
    y    = relu(Hn) @ W_out + b_out         # rows [0, 50000)

Only nodes < 50000 reach the output, so only their in-edges matter.
Sharding: 8 cores x 6250 output nodes. Each core gathers pre-scaled
source rows (dinv[s]*x[s]) from a per-core compact table in DRAM via
gpsimd dma_gather (256B rows), does the segmented reduction on DVE in a
degree-sorted node-chunk layout, and runs the gate pipeline on PE/ACT/DVE
in transposed [128 filters x nodes] layout. Host un-permutes the output.
"""
import os
import sys

import numpy as np

for _p in ("/root/.axon_site", "/root/.axon_site/_ro/trn_rl_repo",
           "/root/.axon_site/_ro/pypackages", "/opt/trn_rl_repo"):
    if os.path.isdir(_p) and _p not in sys.path:
        sys.path.append(_p)

N = 100000
E = 400000
DIN = 32
FLT = 128
NP_ = 8
NA = 50000
NCORES = 8
NODES_PER_CORE = NA // NCORES          # 6250
P = 128
NCHUNK = (NODES_PER_CORE + P - 1) // P  # 49
NODES_PAD = NCHUNK * P                  # 6272
ES = 64                                 # table row: 64 f32 = 256B (32 used)
CHUNKS_PER_GROUP = 4

_cache = {}


def _split_sync_waits(nc, mybir, limit=1):
    """walrus CoreV3 codegen supports one sync-wait per instruction."""
    cnt = 0
    for fn in nc.m.functions:
        for bb in fn.blocks:
            insts = list(bb.instructions)
            out = []
            changed = False
            for inst in insts:
                si = inst.sync_info
                if si is not None and si.on_wait is not None and len(si.on_wait) > limit:
                    w = list(si.on_wait)
                    upd = list(si.on_update) if si.on_update else []
                    chunks = [w[i:i + limit] for i in range(0, len(w), limit)]
                    for chunk in chunks[:-1]:
                        d = mybir.InstDrain(name=f"I-wsplit{cnt}", ins=[], outs=[])
                        cnt += 1
                        d.engine = inst.engine
                        d.sync_info = mybir.SyncInfo(on_wait=chunk, on_update=[])
                        out.append(d)
                    inst.sync_info = mybir.SyncInfo(on_wait=chunks[-1], on_update=upd)
                    changed = True
                out.append(inst)
            if changed:
                bb.instructions = out


def _build_device_kernel(kprof, groups, S, T, SIDX):
    """Build the Bass program. kprof[c] = slots per node-chunk c; groups =
    list of lists of chunk ids; S = total slots; T = table rows; SIDX = idx
    cols (S/16)."""
    import concourse.bacc as bacc
    import concourse.mybir as mybir
    from concourse.tile import TileContext
    from concourse import library_config
    from concourse.masks import make_identity

    nc = bacc.Bacc("TRN2", num_swdge_queues=4)
    f32 = mybir.dt.float32
    bf16 = mybir.dt.bfloat16

    xt = nc.declare_dram_parameter("xt", [T, ES], f32, isOutput=False)
    gidx = nc.declare_dram_parameter("gidx", [P, SIDX], mybir.dt.int16, isOutput=False)
    xself = nc.declare_dram_parameter("xself", [P, NCHUNK * DIN], f32, isOutput=False)
    dinvd = nc.declare_dram_parameter("dinvd", [P, NCHUNK], f32, isOutput=False)
    Az = nc.declare_dram_parameter("Az", [DIN, FLT], bf16, isOutput=False)
    Ah = nc.declare_dram_parameter("Ah", [DIN, FLT], bf16, isOutput=False)
    azn = nc.declare_dram_parameter("azn", [FLT, 1], f32, isOutput=False)
    ahb = nc.declare_dram_parameter("ahb", [FLT, 1], f32, isOutput=False)
    wout = nc.declare_dram_parameter("wout", [FLT, NP_], bf16, isOutput=False)
    bout = nc.declare_dram_parameter("bout", [NP_, 1], f32, isOutput=False)
    yout = nc.declare_dram_parameter("y", [NP_, NODES_PER_CORE], f32, isOutput=True)

    # chunk column offsets in slot space
    choff = np.concatenate([[0], np.cumsum(kprof)]).astype(int)

    nc.gpsimd.load_library(library_config.mlp)

    with TileContext(nc) as tc:
        with (
            tc.tile_pool(name="const", bufs=1) as cp,
            tc.tile_pool(name="g", bufs=1) as gp,
            tc.tile_pool(name="xsc", bufs=6) as xcp,
            tc.tile_pool(name="xsb", bufs=1) as xsp,
            tc.tile_pool(name="ps", bufs=2, space="PSUM") as pp,
            tc.tile_pool(name="psy", bufs=2, space="PSUM") as pyp,
            tc.tile_pool(name="act", bufs=3) as ap,
        ):
            # constants
            idx_t = cp.tile([P, SIDX], mybir.dt.int16)
            nc.sync.dma_start(out=idx_t[:], in_=gidx[:, :])
            dinv_t = cp.tile([P, NCHUNK], f32)
            nc.sync.dma_start(out=dinv_t[:], in_=dinvd[:, :])
            xself_t = cp.tile([P, NCHUNK * DIN], f32)
            nc.sync.dma_start(out=xself_t[:], in_=xself[:, :])
            az_t = cp.tile([DIN, FLT], bf16)
            nc.sync.dma_start(out=az_t[:], in_=Az[:, :])
            ah_t = cp.tile([DIN, FLT], bf16)
            nc.sync.dma_start(out=ah_t[:], in_=Ah[:, :])
            azn_t = cp.tile([FLT, 1], f32)
            nc.sync.dma_start(out=azn_t[:], in_=azn[:, :])
            ahb_t = cp.tile([FLT, 1], f32)
            nc.sync.dma_start(out=ahb_t[:], in_=ahb[:, :])
            wout_t = cp.tile([FLT, NP_], bf16)
            nc.sync.dma_start(out=wout_t[:], in_=wout[:, :])
            bout_t = cp.tile([NP_, 1], f32)
            nc.sync.dma_start(out=bout_t[:], in_=bout[:, :])
            ident = cp.tile([P, P], bf16)
            make_identity(nc, ident[:])
            y_sb = cp.tile([NP_, NODES_PAD], f32)

            for gi, chunks in enumerate(groups):
                gslots = sum(kprof[c] for c in chunks) * P
                gbase = choff[chunks[0]] * P  # slot base of group
                ncols = len(chunks) * P
                xsT = xsp.tile([DIN, ncols], bf16, tag=f"xsT{gi}")
                if gslots > 0:
                    gt = gp.tile([P, (gslots // P) * ES], f32, tag=f"g{gi}")
                    nc.gpsimd.dma_gather(
                        gt[:].rearrange("p (k f) -> p k f", f=ES),
                        xt[:, :],
                        idx_t[:, gbase // 16:(gbase + gslots) // 16],
                        gslots, gslots, ES,
                        single_packet=False, queue_num=gi % 4)
                for j, c in enumerate(chunks):
                    K = kprof[c]
                    xs_b = xcp.tile([P, DIN], bf16, tag="xsb")
                    if K > 0:
                        koff = (choff[c] - choff[chunks[0]]) * ES
                        gin = gt[:, koff:koff + K * ES].rearrange(
                            "p (k f) -> p f k", f=ES)[:, :DIN, :]
                        xs_c = xcp.tile([P, DIN], f32, tag="xs")
                        nc.vector.tensor_reduce(
                            out=xs_c[:], in_=gin,
                            axis=mybir.AxisListType.X, op=mybir.AluOpType.add)
                        nc.vector.tensor_scalar_mul(
                            xs_c[:], xs_c[:], dinv_t[:, c:c + 1])
                        nc.vector.tensor_add(
                            out=xs_b[:], in0=xs_c[:],
                            in1=xself_t[:, c * DIN:(c + 1) * DIN])
                    else:
                        nc.vector.tensor_copy(
                            out=xs_b[:], in_=xself_t[:, c * DIN:(c + 1) * DIN])
                    # transpose [128, 32] -> [32, 128]
                    ps_t = pp.tile([DIN, P], bf16, tag="pst")
                    nc.tensor.transpose(out=ps_t[:], in_=xs_b[:], identity=ident[:])
                    nc.scalar.copy(out=xsT[:, j * P:(j + 1) * P], in_=ps_t[:])

                # gate pipeline for this group's columns
                uz = pp.tile([FLT, ncols], f32, tag="uz")
                uh = pp.tile([FLT, ncols], f32, tag="uh")
                nc.tensor.matmul(out=uz[:], lhsT=az_t[:], rhs=xsT[:], start=True, stop=True)
                nc.tensor.matmul(out=uh[:], lhsT=ah_t[:], rhs=xsT[:], start=True, stop=True)
                zc = ap.tile([FLT, ncols], bf16, tag="zc")
                ht = ap.tile([FLT, ncols], bf16, tag="ht")
                nc.scalar.activation(
                    out=zc[:], in_=uz[:],
                    func=mybir.ActivationFunctionType.Sigmoid,
                    bias=azn_t[:, :1], scale=-1.0)
                nc.scalar.activation(
                    out=ht[:], in_=uh[:],
                    func=mybir.ActivationFunctionType.Tanh,
                    bias=ahb_t[:, :1], scale=1.0)
                pr = ap.tile([FLT, ncols], bf16, tag="pr")
                nc.vector.tensor_mul(out=pr[:], in0=zc[:], in1=ht[:])
                nc.vector.tensor_scalar_max(pr[:], pr[:], 0.0)
                yp = pyp.tile([NP_, ncols], f32, tag="yp")
                nc.tensor.matmul(out=yp[:], lhsT=wout_t[:], rhs=pr[:], start=True, stop=True)
                col0 = chunks[0] * P
                nc.scalar.activation(
                    out=y_sb[:, col0:col0 + ncols], in_=yp[:],
                    func=mybir.ActivationFunctionType.Identity,
                    bias=bout_t[:, :1], scale=1.0)

            nc.sync.dma_start(out=yout[:, :], in_=y_sb[:, :NODES_PER_CORE])

    import concourse.mybir as mybir2
    _split_sync_waits(nc, mybir2)
    nc.compile()
    return nc


def _numpy_fallback(x, H, edge_index, Wz, bz, Wr, br, Wh, bh,
                    Lz_w, Lz_b, Lr_w, Lr_b, Lh_w, Lh_b, W_out, b_out):
    """Exact replica of the reference for unexpected inputs (H != 0)."""
    src = np.asarray(edge_index[0], dtype=np.int64)
    dst = np.asarray(edge_index[1], dtype=np.int64)
    deg = np.zeros(N, np.float32)
    np.add.at(deg, dst, 1.0)
    deg += 1.0
    dinv = (1.0 / np.sqrt(deg)).astype(np.float32)

    def gcn(W, b):
        h = x @ W
        norm = (dinv[src] * dinv[dst]).astype(np.float32)
        agg = np.zeros_like(h)
        np.add.at(agg, dst, h[src] * norm[:, None])
        agg = agg + h * (dinv * dinv)[:, None]
        return agg + b

    def sigmoid(v):
        return 1.0 / (1.0 + np.exp(-v))

    cz = gcn(Wz, bz)
    cr = gcn(Wr, br)
    ch = gcn(Wh, bh)
    Z = sigmoid(np.concatenate([cz, H], axis=1) @ Lz_w + Lz_b)
    R = sigmoid(np.concatenate([cr, H], axis=1) @ Lr_w + Lr_b)
    Ht = np.tanh(np.concatenate([ch, H * R], axis=1) @ Lh_w + Lh_b)
    Hn = Z * H + (1.0 - Z) * Ht
    y = np.maximum(Hn, 0.0) @ W_out + b_out
    return y[:NA].astype(np.float32)


def kernel(x, H, edge_index, Wz, bz, Wr, br, Wh, bh,
           Lz_w, Lz_b, Lr_w, Lr_b, Lh_w, Lh_b, W_out, b_out):
    x = np.asarray(x, dtype=np.float32)
    H = np.asarray(H)
    if H.size and np.any(H):
        return _numpy_fallback(x, np.asarray(H, np.float32), edge_index,
                               np.asarray(Wz, np.float32), np.asarray(bz, np.float32),
                               np.asarray(Wr, np.float32), np.asarray(br, np.float32),
                               np.asarray(Wh, np.float32), np.asarray(bh, np.float32),
                               np.asarray(Lz_w, np.float32), np.asarray(Lz_b, np.float32),
                               np.asarray(Lr_w, np.float32), np.asarray(Lr_b, np.float32),
                               np.asarray(Lh_w, np.float32), np.asarray(Lh_b, np.float32),
                               np.asarray(W_out, np.float32), np.asarray(b_out, np.float32))

    src = np.asarray(edge_index[0], dtype=np.int64)
    dst = np.asarray(edge_index[1], dtype=np.int64)

    # --- normalization (host: integer counts + O(N) scalar table) ---
    deg = np.bincount(dst, minlength=N).astype(np.float32) + 1.0
    dinv = (1.0 / np.sqrt(deg)).astype(np.float32)

    # --- folded gate weights (H = 0 path) ---
    Wz = np.asarray(Wz, np.float32); Wh = np.asarray(Wh, np.float32)
    Lz_top = np.asarray(Lz_w, np.float32)[:FLT]
    Lh_top = np.asarray(Lh_w, np.float32)[:FLT]
    import ml_dtypes
    bf = ml_dtypes.bfloat16
    Az = (Wz @ Lz_top).astype(bf)                               # [32,128]
    Ah = (Wh @ Lh_top).astype(bf)
    az = (np.asarray(bz, np.float32) @ Lz_top + np.asarray(Lz_b, np.float32)).astype(np.float32)
    ah = (np.asarray(bh, np.float32) @ Lh_top + np.asarray(Lh_b, np.float32)).astype(np.float32)
    Wout = np.asarray(W_out, np.float32).astype(bf)             # [128,8]
    bout = np.asarray(b_out, np.float32)                        # [8]

    # --- live edges: only dst < NA contribute to the output ---
    live = dst < NA
    srcL = src[live]
    dstL = dst[live]

    # per-core packing
    per_core = []
    counts_sorted_all = np.empty((NCORES, NODES_PAD), np.int64)
    for c in range(NCORES):
        lo, hi = c * NODES_PER_CORE, (c + 1) * NODES_PER_CORE
        m = (dstL >= lo) & (dstL < hi)
        s_c = srcL[m]
        d_c = dstL[m] - lo
        cnt = np.bincount(d_c, minlength=NODES_PER_CORE)
        perm = np.argsort(-cnt, kind="stable")
        cs = np.zeros(NODES_PAD, np.int64)
        cs[:NODES_PER_CORE] = cnt[perm]
        counts_sorted_all[c] = cs
        per_core.append((s_c, d_c, cnt, perm))

    # uniform per-chunk slot profile across cores
    kprof = np.zeros(NCHUNK, np.int64)
    for ci in range(NCHUNK):
        kprof[ci] = counts_sorted_all[:, ci * P:(ci + 1) * P].max()
    # groups of consecutive chunks (aligned with 512-col matmul blocks)
    groups = [list(range(g, min(g + CHUNKS_PER_GROUP, NCHUNK)))
              for g in range(0, NCHUNK, CHUNKS_PER_GROUP)]
    S = int(kprof.sum()) * P
    # idx wrap granularity: each group's slot range must align to 16 cols
    assert S % 16 == 0
    SIDX = S // 16

    # per-core tables and index arrays
    uniq_list, rows_list = [], []
    for c in range(NCORES):
        s_c, _, _, _ = per_core[c]
        uniq = np.unique(s_c)
        uniq_list.append(uniq)
        rows_list.append(len(uniq) + 1)
    T = int(max(rows_list))

    in_maps = []
    perms = []
    choff = np.concatenate([[0], np.cumsum(kprof)]).astype(np.int64)
    for c in range(NCORES):
        s_c, d_c, cnt, perm = per_core[c]
        uniq = uniq_list[c]
        # compact pre-scaled table: row 0 = zeros
        tab = np.zeros((T, ES), np.float32)
        tab[1:len(uniq) + 1, :DIN] = x[uniq] * dinv[uniq][:, None]
        # per-node padded slot lists in table-row space
        Kmax = int(kprof.max())
        slot = np.zeros((NODES_PAD, Kmax), np.int16)
        row_of = np.searchsorted(uniq, s_c) + 1
        order = np.argsort(d_c, kind="stable")
        d_s = d_c[order]
        r_s = row_of[order]
        starts = np.zeros(NODES_PER_CORE + 1, np.int64)
        np.cumsum(cnt, out=starts[1:])
        within = np.arange(len(d_s)) - starts[d_s]
        slot[d_s, within] = r_s.astype(np.int16)
        slot_perm = np.zeros((NODES_PAD, Kmax), np.int16)
        slot_perm[:NODES_PER_CORE] = slot[perm]
        # flat slot order: chunk-major, then k, then partition
        flat = np.zeros(S, np.int16)
        for ci in range(NCHUNK):
            K = int(kprof[ci])
            if K == 0:
                continue
            blk = slot_perm[ci * P:(ci + 1) * P, :K]      # [128, K]
            flat[choff[ci] * P:(choff[ci] + K) * P] = blk.T.reshape(-1)
        wrapped = np.tile(flat.reshape(SIDX, 16).T, (8, 1))   # [128, SIDX]
        # per-node scalars in perm order
        nodes_perm = perm + c * NODES_PER_CORE
        dv = np.ones(NODES_PAD, np.float32)
        dv[:NODES_PER_CORE] = dinv[nodes_perm]
        dinvd = dv.reshape(NCHUNK, P).T.copy()                # [128, NCHUNK]
        xs_self = np.zeros((NODES_PAD, DIN), np.float32)
        xs_self[:NODES_PER_CORE] = x[nodes_perm] * (dinv[nodes_perm] ** 2)[:, None]
        xs_self = np.ascontiguousarray(
            xs_self.reshape(NCHUNK, P, DIN).transpose(1, 0, 2).reshape(P, NCHUNK * DIN))
        perms.append(perm)
        in_maps.append({
            "xt": tab, "gidx": wrapped, "xself": xs_self, "dinvd": dinvd,
            "Az": Az, "Ah": Ah, "azn": (-az).reshape(FLT, 1),
            "ahb": ah.reshape(FLT, 1), "wout": Wout,
            "bout": bout.reshape(NP_, 1),
        })

    if os.environ.get("KERNEL_DEBUG") == "1":
        print(f"[kernel] S={S} slots ({S/NCORES:.0f}/... total pad "
              f"{S - len(srcL)//NCORES}) T={T} kprof={kprof.tolist()}")
    key = ("v2", tuple(kprof.tolist()), S, T)
    if key not in _cache:
        _cache[key] = _build_device_kernel(kprof, groups, S, T, SIDX)
    nc = _cache[key]

    from concourse.bass_utils import run_bass_kernel_spmd
    trace = os.environ.get("KERNEL_TRACE") == "1"
    kwargs = {}
    if trace:
        kwargs = {"trace": True, "tmpdir": os.environ.get("KERNEL_TRACE_DIR", "/tmp/kernel_trace")}
    res = run_bass_kernel_spmd(nc, in_maps, list(range(NCORES)), **kwargs)
    global last_result
    last_result = res

    y = np.empty((NA, NP_), np.float32)
    for c in range(NCORES):
        yc = res.results[c]["y"]                      # [8, 6250] in perm order
        lo = c * NODES_PER_CORE
        y[lo + perms[c], :] = yc.T
    return y
